# revision 26
# baseline (speedup 1.0000x reference)
"""Trainium2 Bass kernel for CustomAttention (B=4, S=2048, D=1024, H=16).

Sharding: 8 cores = 4 batches x 2 head-halves (8 heads each). Each core
computes Q/K/V projections for its 512 head-dims, attention for its 8 heads
over all 2048 queries, and a partial out-projection (contraction over its 512
dims). Host sums the two partial outputs per batch; bo/2 is added on each core
so the host sum carries the full bias.

Performance structure:
  - All matmul operands 16-bit (fp16 where the value range allows, bf16 where
    exp magnitudes flow: pt, v_pad, unnormalized attention). Full-rate PE.
  - softmax scale folded into Wq/bq host-side.
  - K^T/Q^T/attention SBUF-resident; inputs staged in [128,512] chunks.
  - Pipelined emission: K proj -> Q proj (pair-major, so attention starts as
    soon as pair 0's Q lands) -> V proj -> attention (qc-outer, pair-inner)
    with per-qc deferred normalization and out-projection chunks inlined every
    other qc. ScalarE exp is the critical engine; everything else hides under.
  - exp in {6,6,4}-kti segments (1536-elem calls amortize ACT fixed overhead,
    3 PSUM banks each, double buffered). PV for both heads accumulates into
    ONE PSUM bank ([65, 2, 256]; sequential per-head chains), ones-column of
    v_pad gives the softmax denominators; one batched reciprocal per qc.
  - mask / key_padding_mask are all-ones for this problem's inputs => identity;
    a numpy fallback handles the (never-hit) general case.
"""

import math

import numpy as np

B, S, D = 4, 2048, 1024
H, DH = 16, 64       # global heads
HL = 8               # local heads per core
P = 128
NPAIR = HL // 2      # 4 local head pairs
NKT = S // P         # 16 key tiles
QC = 256             # query chunk for attention
NQC = S // QC        # 8
DL = 512             # local projection width (8 heads x 64)
SCALE = math.log(D) / math.sqrt(DH)
SEGS = [(0, 6), (6, 6), (12, 4)]  # kti segments for QK/exp

_CACHE = {}


def _build_nc():
    import concourse.bass as bass
    import concourse.bacc as bacc
    import concourse.mybir as mybir
    import concourse.tile as tile
    from contextlib import ExitStack

    f32 = mybir.dt.float32
    f16 = mybir.dt.float16
    bf16 = mybir.dt.bfloat16
    EXP = mybir.ActivationFunctionType.Exp
    ADD = mybir.AluOpType.add
    MULT = mybir.AluOpType.mult

    nc = bacc.Bacc("TRN2", target_bir_lowering=False, debug=False, num_devices=8)

    queryT = nc.declare_dram_parameter("queryT", [D, S], f16, isOutput=False)
    keyT = nc.declare_dram_parameter("keyT", [D, S], f16, isOutput=False)
    valueT = nc.declare_dram_parameter("valueT", [D, S], f16, isOutput=False)
    WqT = nc.declare_dram_parameter("WqT", [D, DL], f16, isOutput=False)
    WkT = nc.declare_dram_parameter("WkT", [D, DL], f16, isOutput=False)
    WvT = nc.declare_dram_parameter("WvT", [D, DL], f16, isOutput=False)
    WoT = nc.declare_dram_parameter("WoT", [DL, D], bf16, isOutput=False)
    bq_d = nc.declare_dram_parameter("bq", [DL], f32, isOutput=False)
    bk_d = nc.declare_dram_parameter("bk", [DL], f32, isOutput=False)
    bv_d = nc.declare_dram_parameter("bv", [1, DL], f32, isOutput=False)
    bo_d = nc.declare_dram_parameter("bo_half", [D], f32, isOutput=False)
    outT = nc.declare_dram_parameter("outT", [D, S], f32, isOutput=True)

    with ExitStack() as ctx:
        tc = ctx.enter_context(tile.TileContext(nc))
        persist = ctx.enter_context(tc.tile_pool(name="persist", bufs=1))
        wpool = ctx.enter_context(tc.tile_pool(name="wpool", bufs=1))
        in512 = ctx.enter_context(tc.tile_pool(name="in512", bufs=18))
        ptp = ctx.enter_context(tc.tile_pool(name="ptp", bufs=9))
        bcp = ctx.enter_context(tc.tile_pool(name="bcp", bufs=4))
        stgp = ctx.enter_context(tc.tile_pool(name="stgp", bufs=2))
        ost = ctx.enter_context(tc.tile_pool(name="ost", bufs=2))
        dnp = ctx.enter_context(tc.tile_pool(name="dnp", bufs=2))
        rcp = ctx.enter_context(tc.tile_pool(name="rcp", bufs=2))
        et = ctx.enter_context(tc.tile_pool(name="et", bufs=2, space="PSUM"))
        pvp = ctx.enter_context(tc.tile_pool(name="pvp", bufs=1, space="PSUM"))
        acc = ctx.enter_context(tc.tile_pool(name="acc", bufs=1, space="PSUM"))
        dram = ctx.enter_context(tc.tile_pool(name="dram", bufs=2, space="DRAM"))

        kT = [
            persist.tile([P, S], f16, tag=f"kT{j}", name=f"kT{j}")
            for j in range(NPAIR)
        ]
        qt = [
            persist.tile([P, S], f16, tag=f"qt{j}", name=f"qt{j}")
            for j in range(NPAIR)
        ]
        attn = [
            persist.tile([P, S], bf16, tag=f"at{j}", name=f"at{j}")
            for j in range(NPAIR)
        ]
        v_pad = persist.tile([P, NKT, HL, DH + 1], bf16, tag="v_pad")
        bq_sb = persist.tile([P, NPAIR], f32, tag="bq")
        bk_sb = persist.tile([P, NPAIR], f32, tag="bk")
        bo_sb = persist.tile([P, 8], f32, tag="bo")
        bv_bc = persist.tile([P, DL], f32, tag="bv_bc")

        # --- setup ---
        nc.sync.dma_start(bq_sb[:], bq_d.rearrange("(o p) -> p o", p=P))
        nc.sync.dma_start(bk_sb[:], bk_d.rearrange("(o p) -> p o", p=P))
        nc.sync.dma_start(bo_sb[:], bo_d.rearrange("(o p) -> p o", p=P))
        nc.sync.dma_start(bv_bc[:], bv_d[:].to_broadcast([P, DL]))
        nc.vector.memset(v_pad[:], 1.0)

        # weights DMA'd lazily at first use so k-input chunks go first
        wk = wpool.tile([P, 8, DL], f16, tag="w", name="wk")
        nc.sync.dma_start(wk[:], WkT[:].rearrange("(k p) c -> p k c", p=P))
        wq = wpool.tile([P, 8, DL], f16, tag="w2", name="wq")
        wv = wpool.tile([P, 8, DL], f16, tag="w3", name="wv")
        wo = wpool.tile([P, NPAIR, D], bf16, tag="w4", name="wo")

        def chunk(srcT, kt, sc, name):
            t = in512.tile([P, 512], f16, tag="in", name=name)
            nc.sync.dma_start(
                t[:], srcT[kt * P:(kt + 1) * P, sc * 512:(sc + 1) * 512]
            )
            return t

        def proj_psum(i):
            # alternate between the two 1-bank pools for 2-deep pipelining
            pool = acc if i % 2 == 0 else pvp
            return pool.tile([P, 512], f32, tag="acc" if i % 2 == 0 else "pv", name=f"pp{i}")

        # --- K projection: kT[j][p(2h x 64dh), seq], sc-major ---
        pi = 0
        for sc in range(4):
            kc = [chunk(keyT, kt, sc, f"kc{kt}_{sc}") for kt in range(8)]
            for j in range(NPAIR):
                ps = proj_psum(pi); pi += 1
                for kt in range(8):
                    nc.tensor.matmul(
                        out=ps[:],
                        lhsT=wk[:, kt, j * P:(j + 1) * P],
                        rhs=kc[kt][:],
                        start=(kt == 0), stop=(kt == 7),
                    )
                nc.vector.tensor_scalar_add(
                    kT[j][:, sc * 512:(sc + 1) * 512], ps[:], bk_sb[:, j:j + 1]
                )

        # --- Q projection emitter (one sc chunk of all pairs at a time) ---
        def q_proj_sc(sc):
            nonlocal pi
            if sc == 0:
                nc.sync.dma_start(wq[:], WqT[:].rearrange("(k p) c -> p k c", p=P))
            qc_ = [chunk(queryT, kt, sc, f"qc{kt}_{sc}") for kt in range(8)]
            for j in range(NPAIR):
                ps = proj_psum(pi); pi += 1
                for kt in range(8):
                    nc.tensor.matmul(
                        out=ps[:],
                        lhsT=wq[:, kt, j * P:(j + 1) * P],
                        rhs=qc_[kt][:],
                        start=(kt == 0), stop=(kt == 7),
                    )
                nc.vector.tensor_scalar_add(
                    qt[j][:, sc * 512:(sc + 1) * 512], ps[:], bq_sb[:, j:j + 1]
                )

        # --- V projection emitter: one pair-group (2 pairs = 256 dims) pass.
        # Split so PV of early pairs can start before all of V is projected. ---
        def v_proj_half(g):
            nonlocal pi
            if g == 0:
                nc.sync.dma_start(wv[:], WvT[:].rearrange("(k p) c -> p k c", p=P))
            for stg4 in range(4):
                vc = [
                    chunk(valueT, kt, stg4, f"vc{g}_{kt}_{stg4}")
                    for kt in range(8)
                ]
                for stl in range(4):
                    st = stg4 * 4 + stl
                    ps = proj_psum(pi); pi += 1
                    for kt in range(8):
                        nc.tensor.matmul(
                            out=ps[:, 0:256],
                            lhsT=vc[kt][:, stl * P:(stl + 1) * P],
                            rhs=wv[:, kt, g * 256:(g + 1) * 256],
                            start=(kt == 0), stop=(kt == 7),
                        )
                    nc.vector.tensor_tensor(
                        v_pad[:, st, g * 4:(g + 1) * 4, 0:DH],
                        ps[:, 0:256].rearrange("p (h d) -> p h d", h=4),
                        bv_bc[:, g * 256:(g + 1) * 256].rearrange(
                            "p (h d) -> p h d", h=4
                        ),
                        ADD,
                    )

        # --- attention emitters: QK+exp and (staggered) PV+normalization ---
        pt_of = {}
        dnq_of = {}

        def att_qk(qc, j):
            pt = [
                ptp.tile([P, NKT, QC], bf16, tag="pt", name=f"pt{qc}_{j}_{h}")
                for h in range(2)
            ]
            pt_of[(qc, j)] = pt
            for sb, sl in SEGS:
                ett = [
                    et.tile([P, 6, QC], f32, tag="et", name=f"et{h}")
                    for h in range(2)
                ]
                for t_i in range(sl):
                    kti = sb + t_i
                    for h in range(2):
                        nc.tensor.matmul(
                            out=ett[h][:, t_i, :],
                            lhsT=kT[j][h * 64:(h + 1) * 64, kti * P:(kti + 1) * P],
                            rhs=qt[j][h * 64:(h + 1) * 64, qc * QC:(qc + 1) * QC],
                            start=True, stop=True,
                            tile_position=(h * 64, 0),
                        )
                for h in range(2):
                    nc.scalar.activation(
                        pt[h][:, sb:sb + sl, :], ett[h][:, 0:sl, :], EXP
                    )

        pv_done = {qc: 0 for qc in range(NQC)}

        def att_pv(qc, j):
            if qc not in dnq_of:
                dnq_of[qc] = dnp.tile(
                    [2 * NPAIR, QC], f32, tag="dn", name=f"dn{qc}"
                )
            dnq = dnq_of[qc]
            pt = pt_of.pop((qc, j))
            pvt = pvp.tile([DH + 1, 2, QC], f32, tag="pv", name=f"pv{qc}_{j}")
            for h in range(2):
                for kti in range(NKT):
                    nc.tensor.matmul(
                        out=pvt[0:DH + 1, h, :],
                        lhsT=v_pad[:, kti, 2 * j + h, 0:DH + 1],
                        rhs=pt[h][:, kti, :],
                        start=(kti == 0), stop=(kti == NKT - 1),
                    )
            stg = stgp.tile([P, 2, QC], f32, tag="dnst")
            for h in range(2):
                nc.vector.tensor_copy(
                    out=attn[j][h * 64:(h + 1) * 64, qc * QC:(qc + 1) * QC],
                    in_=pvt[0:DH, h, :],
                )
                nc.vector.tensor_copy(
                    out=stg[64:65, h, :], in_=pvt[DH:DH + 1, h, :]
                )
            nc.sync.dma_start(dnq[2 * j:2 * j + 2, :], stg[64:65, :, :])
            pv_done[qc] += 1
            if pv_done[qc] == NPAIR:
                norm(qc)
                o_proj_q(qc)

        def norm(qc):
            dnq = dnq_of.pop(qc)
            rcq = rcp.tile([2 * NPAIR, QC], f32, tag="rc", name=f"rc{qc}")
            nc.vector.reciprocal(rcq[:], dnq[:])
            rcd = dram.tile([NPAIR * 2, QC], f32, tag="rcd", name=f"rcd{qc}")
            nc.sync.dma_start(rcd[:], rcq[:])
            for j in range(NPAIR):
                bc_t = bcp.tile([P, QC], f32, tag="bc")
                nc.sync.dma_start(
                    bc_t[0:64, :],
                    rcd[2 * j:2 * j + 1, :].to_broadcast([64, QC]),
                )
                nc.sync.dma_start(
                    bc_t[64:P, :],
                    rcd[2 * j + 1:2 * j + 2, :].to_broadcast([64, QC]),
                )
                nc.vector.tensor_tensor(
                    attn[j][:, qc * QC:(qc + 1) * QC],
                    attn[j][:, qc * QC:(qc + 1) * QC],
                    bc_t[:],
                    MULT,
                )

        def o_proj_q(qc):
            if qc == 0:
                nc.sync.dma_start(wo[:], WoT[:].rearrange("(c p) d -> p c d", p=P))
            if qc >= NQC - 2:
                c0, cw = qc * QC, QC       # last two qc: drain per-chunk
            elif qc % 2 == 1:
                c0, cw = (qc // 2) * 512, 512
            else:
                return
            for dt in range(8):
                ps = acc.tile([P, 512], f32, tag="acc", name=f"op{qc}_{dt}")
                for ct in range(NPAIR):
                    nc.tensor.matmul(
                        out=ps[:, 0:cw],
                        lhsT=wo[:, ct, dt * P:(dt + 1) * P],
                        rhs=attn[ct][:, c0:c0 + cw],
                        start=(ct == 0), stop=(ct == NPAIR - 1),
                    )
                o_t = ost.tile([P, 512], f32, tag="ost")
                nc.vector.tensor_scalar_add(
                    o_t[:, 0:cw], ps[:, 0:cw], bo_sb[:, dt:dt + 1]
                )
                nc.sync.dma_start(
                    outT[dt * P:(dt + 1) * P, c0:c0 + cw], o_t[:, 0:cw]
                )

        # --- pipelined emission: PV lags QK by 3 units. Unit order runs
        # pairs {0,1} of qc0/qc1 first so PV can start after only the first
        # V half-pass; pairs {2,3} follow once the second half lands. Q sc1-3
        # and the V passes fill PE slack under the ACT-bound stream. ---
        units = [(qc, j) for qc in range(NQC) for j in range(NPAIR)]
        LAG = 3
        q_proj_sc(0)
        fills = {
            3: lambda: v_proj_half(0),   # before pv(qc0,j0): pairs 0-1
            4: lambda: q_proj_sc(1),
            5: lambda: v_proj_half(1),   # before pv(qc0,j2): pairs 2-3
            9: lambda: q_proj_sc(2),
            13: lambda: q_proj_sc(3),
        }
        n_units = len(units)
        pv_next = 0
        for u in range(n_units):
            if u in fills:
                fills[u]()
            att_qk(*units[u])
            # taper the PV lag near the end so the pipeline drains early
            lag = LAG if u < n_units - 5 else max(1, n_units - 1 - u)
            while pv_next <= u - lag:
                att_pv(*units[pv_next])
                pv_next += 1
        while pv_next < n_units:
            att_pv(*units[pv_next])
            pv_next += 1

    if not nc.is_finalized():
        nc.finalize()
    return nc


def get_nc():
    if "nc" not in _CACHE:
        _CACHE["nc"] = _build_nc()
    return _CACHE["nc"]


def make_in_maps(inputs):
    f16 = np.float16
    import ml_dtypes

    bf16 = ml_dtypes.bfloat16
    q = np.asarray(inputs["query"], np.float32)
    k = np.asarray(inputs["key"], np.float32)
    v = np.asarray(inputs["value"], np.float32)
    Wq = np.asarray(inputs["Wq"], np.float32) * SCALE  # fold softmax scale
    Wk = np.asarray(inputs["Wk"], np.float32)
    Wv = np.asarray(inputs["Wv"], np.float32)
    Wo = np.asarray(inputs["Wo"], np.float32)
    bq = np.asarray(inputs["bq"], np.float32) * SCALE
    bk = np.asarray(inputs["bk"], np.float32)
    bv = np.asarray(inputs["bv"], np.float32)
    bo_half = np.asarray(inputs["bo"], np.float32) * 0.5

    qT = [np.ascontiguousarray(q[b].T.astype(f16)) for b in range(B)]
    kTl = [np.ascontiguousarray(k[b].T.astype(f16)) for b in range(B)]
    vT = [np.ascontiguousarray(v[b].T.astype(f16)) for b in range(B)]
    WqTs = [np.ascontiguousarray(Wq.T[:, i * DL:(i + 1) * DL].astype(f16)) for i in range(2)]
    WkTs = [np.ascontiguousarray(Wk.T[:, i * DL:(i + 1) * DL].astype(f16)) for i in range(2)]
    WvTs = [np.ascontiguousarray(Wv.T[:, i * DL:(i + 1) * DL].astype(f16)) for i in range(2)]
    WoTs = [np.ascontiguousarray(Wo.T[i * DL:(i + 1) * DL, :].astype(bf16)) for i in range(2)]
    bqs = [np.ascontiguousarray(bq[i * DL:(i + 1) * DL]) for i in range(2)]
    bks = [np.ascontiguousarray(bk[i * DL:(i + 1) * DL]) for i in range(2)]
    bvs = [np.ascontiguousarray(bv[i * DL:(i + 1) * DL]).reshape(1, DL) for i in range(2)]

    in_maps = []
    for c in range(8):
        b, hh = c // 2, c % 2
        in_maps.append({
            "queryT": qT[b], "keyT": kTl[b], "valueT": vT[b],
            "WqT": WqTs[hh], "WkT": WkTs[hh], "WvT": WvTs[hh], "WoT": WoTs[hh],
            "bq": bqs[hh], "bk": bks[hh], "bv": bvs[hh], "bo_half": bo_half,
        })
    return in_maps


def assemble(results):
    out = np.empty((B, S, D), np.float32)
    for b in range(B):
        out[b] = (results[2 * b]["outT"] + results[2 * b + 1]["outT"]).T
    return out


def _numpy_fallback(inputs):
    q = np.asarray(inputs["query"], np.float64)
    k = np.asarray(inputs["key"], np.float64)
    v = np.asarray(inputs["value"], np.float64)
    Wq, bq = np.asarray(inputs["Wq"], np.float64), np.asarray(inputs["bq"], np.float64)
    Wk, bk = np.asarray(inputs["Wk"], np.float64), np.asarray(inputs["bk"], np.float64)
    Wv, bv = np.asarray(inputs["Wv"], np.float64), np.asarray(inputs["bv"], np.float64)
    Wo, bo = np.asarray(inputs["Wo"], np.float64), np.asarray(inputs["bo"], np.float64)
    qp = (q @ Wq.T + bq).reshape(B, S, H, DH).transpose(0, 2, 1, 3)
    kp = (k @ Wk.T + bk).reshape(B, S, H, DH).transpose(0, 2, 1, 3)
    vp = (v @ Wv.T + bv).reshape(B, S, H, DH).transpose(0, 2, 1, 3)
    e = np.einsum("bhqd,bhkd->bhqk", qp, kp) * SCALE
    mask = np.asarray(inputs["mask"])
    kpm = np.asarray(inputs["key_padding_mask"])
    e = np.where(mask == 0, -np.inf, e)
    e = np.where(kpm[:, None, None, :] == 0, -np.inf, e)
    e -= e.max(axis=-1, keepdims=True)
    p = np.exp(e)
    p /= p.sum(axis=-1, keepdims=True)
    o = np.einsum("bhqk,bhkd->bhqd", p, vp).transpose(0, 2, 1, 3).reshape(B, S, D)
    return (o @ Wo.T + bo).astype(np.float32)


def kernel(**inputs):
    mask = np.asarray(inputs["mask"])
    kpm = np.asarray(inputs["key_padding_mask"])
    if not (mask.all() and kpm.all()):
        return _numpy_fallback(inputs)
    from concourse.bass_utils import run_bass_kernel_spmd

    nc = get_nc()
    in_maps = make_in_maps(inputs)
    res = run_bass_kernel_spmd(nc, in_maps, list(range(8)))
    return assemble(res.results)


# revision 27
# speedup vs baseline: 1.0214x; 1.0214x over previous
"""Trainium2 Bass kernel for CustomAttention (B=4, S=2048, D=1024, H=16).

Sharding: 8 cores = 4 batches x 2 head-halves (8 heads each). Each core
computes Q/K/V projections for its 512 head-dims, attention for its 8 heads
over all 2048 queries, and a partial out-projection (contraction over its 512
dims). Host sums the two partial outputs per batch; bo/2 is added on each core
so the host sum carries the full bias.

Performance structure:
  - All matmul operands 16-bit (fp16 where the value range allows, bf16 where
    exp magnitudes flow: pt, v_pad, unnormalized attention). Full-rate PE.
  - softmax scale folded into Wq/bq host-side.
  - K^T/Q^T/attention SBUF-resident; inputs staged in [128,512] chunks.
  - Pipelined emission: K proj -> Q proj (pair-major, so attention starts as
    soon as pair 0's Q lands) -> V proj -> attention (qc-outer, pair-inner)
    with per-qc deferred normalization and out-projection chunks inlined every
    other qc. ScalarE exp is the critical engine; everything else hides under.
  - exp in {6,6,4}-kti segments (1536-elem calls amortize ACT fixed overhead,
    3 PSUM banks each, double buffered). PV for both heads accumulates into
    ONE PSUM bank ([65, 2, 256]; sequential per-head chains), ones-column of
    v_pad gives the softmax denominators; one batched reciprocal per qc.
  - mask / key_padding_mask are all-ones for this problem's inputs => identity;
    a numpy fallback handles the (never-hit) general case.
"""

import math

import numpy as np

B, S, D = 4, 2048, 1024
H, DH = 16, 64       # global heads
HL = 8               # local heads per core
P = 128
NPAIR = HL // 2      # 4 local head pairs
NKT = S // P         # 16 key tiles
QC = 256             # query chunk for attention
NQC = S // QC        # 8
DL = 512             # local projection width (8 heads x 64)
SCALE = math.log(D) / math.sqrt(DH)
SEGS = [(0, 6), (6, 6), (12, 4)]  # kti segments for QK/exp

_CACHE = {}


def _build_nc():
    import concourse.bass as bass
    import concourse.bacc as bacc
    import concourse.mybir as mybir
    import concourse.tile as tile
    from contextlib import ExitStack

    f32 = mybir.dt.float32
    f16 = mybir.dt.float16
    bf16 = mybir.dt.bfloat16
    EXP = mybir.ActivationFunctionType.Exp
    ADD = mybir.AluOpType.add
    MULT = mybir.AluOpType.mult

    nc = bacc.Bacc("TRN2", target_bir_lowering=False, debug=False, num_devices=8)

    queryT = nc.declare_dram_parameter("queryT", [D, S], f16, isOutput=False)
    keyT = nc.declare_dram_parameter("keyT", [D, S], f16, isOutput=False)
    valueT = nc.declare_dram_parameter("valueT", [D, S], f16, isOutput=False)
    WqT = nc.declare_dram_parameter("WqT", [D, DL], f16, isOutput=False)
    WkT = nc.declare_dram_parameter("WkT", [D, DL], f16, isOutput=False)
    WvT = nc.declare_dram_parameter("WvT", [D, DL], f16, isOutput=False)
    WoT = nc.declare_dram_parameter("WoT", [DL, D], bf16, isOutput=False)
    bq_d = nc.declare_dram_parameter("bq", [DL], f32, isOutput=False)
    bk_d = nc.declare_dram_parameter("bk", [DL], f32, isOutput=False)
    bv_d = nc.declare_dram_parameter("bv", [1, DL], f32, isOutput=False)
    bo_d = nc.declare_dram_parameter("bo_half", [D], f32, isOutput=False)
    outT = nc.declare_dram_parameter("outT", [D, S], f32, isOutput=True)

    with ExitStack() as ctx:
        tc = ctx.enter_context(tile.TileContext(nc))
        persist = ctx.enter_context(tc.tile_pool(name="persist", bufs=1))
        wpool = ctx.enter_context(tc.tile_pool(name="wpool", bufs=1))
        in512 = ctx.enter_context(tc.tile_pool(name="in512", bufs=24))
        ptp = ctx.enter_context(tc.tile_pool(name="ptp", bufs=8))
        bcp = ctx.enter_context(tc.tile_pool(name="bcp", bufs=4))
        stgp = ctx.enter_context(tc.tile_pool(name="stgp", bufs=2))
        ost = ctx.enter_context(tc.tile_pool(name="ost", bufs=2))
        dnp = ctx.enter_context(tc.tile_pool(name="dnp", bufs=2))
        rcp = ctx.enter_context(tc.tile_pool(name="rcp", bufs=2))
        et = ctx.enter_context(tc.tile_pool(name="et", bufs=2, space="PSUM"))
        pvp = ctx.enter_context(tc.tile_pool(name="pvp", bufs=1, space="PSUM"))
        acc = ctx.enter_context(tc.tile_pool(name="acc", bufs=1, space="PSUM"))
        dram = ctx.enter_context(tc.tile_pool(name="dram", bufs=2, space="DRAM"))

        kT = [
            persist.tile([P, S], f16, tag=f"kT{j}", name=f"kT{j}")
            for j in range(NPAIR)
        ]
        qt = [
            persist.tile([P, S], f16, tag=f"qt{j}", name=f"qt{j}")
            for j in range(NPAIR)
        ]
        attn = [
            persist.tile([P, S], bf16, tag=f"at{j}", name=f"at{j}")
            for j in range(NPAIR)
        ]
        v_pad = persist.tile([P, NKT, HL, DH + 1], bf16, tag="v_pad")
        bq_sb = persist.tile([P, NPAIR], f32, tag="bq")
        bk_sb = persist.tile([P, NPAIR], f32, tag="bk")
        bo_sb = persist.tile([P, 8], f32, tag="bo")
        bv_bc = persist.tile([P, DL], f32, tag="bv_bc")

        # --- setup ---
        nc.sync.dma_start(bq_sb[:], bq_d.rearrange("(o p) -> p o", p=P))
        nc.sync.dma_start(bk_sb[:], bk_d.rearrange("(o p) -> p o", p=P))
        nc.sync.dma_start(bo_sb[:], bo_d.rearrange("(o p) -> p o", p=P))
        nc.sync.dma_start(bv_bc[:], bv_d[:].to_broadcast([P, DL]))
        nc.vector.memset(v_pad[:], 1.0)

        # weights DMA'd lazily at first use so k-input chunks go first
        wk = wpool.tile([P, 8, DL], f16, tag="w", name="wk")
        nc.sync.dma_start(wk[:], WkT[:].rearrange("(k p) c -> p k c", p=P))
        wq = wpool.tile([P, 8, DL], f16, tag="w2", name="wq")
        wv = wpool.tile([P, 8, DL], f16, tag="w3", name="wv")
        wo = wpool.tile([P, NPAIR, D], bf16, tag="w4", name="wo")

        def chunk(srcT, kt, sc, name):
            t = in512.tile([P, 512], f16, tag="in", name=name)
            nc.sync.dma_start(
                t[:], srcT[kt * P:(kt + 1) * P, sc * 512:(sc + 1) * 512]
            )
            return t

        def proj_psum(i):
            # alternate between the two 1-bank pools for 2-deep pipelining
            pool = acc if i % 2 == 0 else pvp
            return pool.tile([P, 512], f32, tag="acc" if i % 2 == 0 else "pv", name=f"pp{i}")

        # --- K projection: kT[j][p(2h x 64dh), seq], sc-major ---
        pi = 0
        for sc in range(4):
            kc = [chunk(keyT, kt, sc, f"kc{kt}_{sc}") for kt in range(8)]
            for j in range(NPAIR):
                ps = proj_psum(pi); pi += 1
                for kt in range(8):
                    nc.tensor.matmul(
                        out=ps[:],
                        lhsT=wk[:, kt, j * P:(j + 1) * P],
                        rhs=kc[kt][:],
                        start=(kt == 0), stop=(kt == 7),
                    )
                nc.vector.tensor_scalar_add(
                    kT[j][:, sc * 512:(sc + 1) * 512], ps[:], bk_sb[:, j:j + 1]
                )

        # --- Q projection emitter (one sc chunk of all pairs at a time) ---
        def q_proj_sc(sc):
            nonlocal pi
            if sc == 0:
                nc.sync.dma_start(wq[:], WqT[:].rearrange("(k p) c -> p k c", p=P))
            qc_ = [chunk(queryT, kt, sc, f"qc{kt}_{sc}") for kt in range(8)]
            for j in range(NPAIR):
                ps = proj_psum(pi); pi += 1
                for kt in range(8):
                    nc.tensor.matmul(
                        out=ps[:],
                        lhsT=wq[:, kt, j * P:(j + 1) * P],
                        rhs=qc_[kt][:],
                        start=(kt == 0), stop=(kt == 7),
                    )
                nc.vector.tensor_scalar_add(
                    qt[j][:, sc * 512:(sc + 1) * 512], ps[:], bq_sb[:, j:j + 1]
                )

        # --- V projection emitter: one pair-group (2 pairs = 256 dims) pass.
        # Split so PV of early pairs can start before all of V is projected. ---
        def v_proj_half(g):
            nonlocal pi
            if g == 0:
                nc.sync.dma_start(wv[:], WvT[:].rearrange("(k p) c -> p k c", p=P))
            for stg4 in range(4):
                vc = [
                    chunk(valueT, kt, stg4, f"vc{g}_{kt}_{stg4}")
                    for kt in range(8)
                ]
                for stl in range(4):
                    st = stg4 * 4 + stl
                    ps = proj_psum(pi); pi += 1
                    for kt in range(8):
                        nc.tensor.matmul(
                            out=ps[:, 0:256],
                            lhsT=vc[kt][:, stl * P:(stl + 1) * P],
                            rhs=wv[:, kt, g * 256:(g + 1) * 256],
                            start=(kt == 0), stop=(kt == 7),
                        )
                    nc.vector.tensor_tensor(
                        v_pad[:, st, g * 4:(g + 1) * 4, 0:DH],
                        ps[:, 0:256].rearrange("p (h d) -> p h d", h=4),
                        bv_bc[:, g * 256:(g + 1) * 256].rearrange(
                            "p (h d) -> p h d", h=4
                        ),
                        ADD,
                    )

        # --- attention emitters: QK+exp and (staggered) PV+normalization ---
        pt_of = {}
        dnq_of = {}

        def att_qk(qc, j):
            pt = [
                ptp.tile([P, NKT, QC], bf16, tag="pt", name=f"pt{qc}_{j}_{h}")
                for h in range(2)
            ]
            pt_of[(qc, j)] = pt
            for sb, sl in SEGS:
                ett = [
                    et.tile([P, 6, QC], f32, tag="et", name=f"et{h}")
                    for h in range(2)
                ]
                for t_i in range(sl):
                    kti = sb + t_i
                    for h in range(2):
                        nc.tensor.matmul(
                            out=ett[h][:, t_i, :],
                            lhsT=kT[j][h * 64:(h + 1) * 64, kti * P:(kti + 1) * P],
                            rhs=qt[j][h * 64:(h + 1) * 64, qc * QC:(qc + 1) * QC],
                            start=True, stop=True,
                            tile_position=(h * 64, 0),
                        )
                for h in range(2):
                    nc.scalar.activation(
                        pt[h][:, sb:sb + sl, :], ett[h][:, 0:sl, :], EXP
                    )

        pv_done = {qc: 0 for qc in range(NQC)}

        def att_pv(qc, j):
            if qc not in dnq_of:
                dnq_of[qc] = dnp.tile(
                    [2 * NPAIR, QC], f32, tag="dn", name=f"dn{qc}"
                )
            dnq = dnq_of[qc]
            pt = pt_of.pop((qc, j))
            pvt = pvp.tile([DH + 1, 2, QC], f32, tag="pv", name=f"pv{qc}_{j}")
            for h in range(2):
                for kti in range(NKT):
                    nc.tensor.matmul(
                        out=pvt[0:DH + 1, h, :],
                        lhsT=v_pad[:, kti, 2 * j + h, 0:DH + 1],
                        rhs=pt[h][:, kti, :],
                        start=(kti == 0), stop=(kti == NKT - 1),
                    )
            stg = stgp.tile([P, 2, QC], f32, tag="dnst")
            for h in range(2):
                nc.vector.tensor_copy(
                    out=attn[j][h * 64:(h + 1) * 64, qc * QC:(qc + 1) * QC],
                    in_=pvt[0:DH, h, :],
                )
                nc.vector.tensor_copy(
                    out=stg[64:65, h, :], in_=pvt[DH:DH + 1, h, :]
                )
            nc.sync.dma_start(dnq[2 * j:2 * j + 2, :], stg[64:65, :, :])
            pv_done[qc] += 1
            if pv_done[qc] == NPAIR:
                norm(qc)
                o_proj_q(qc)

        def norm(qc):
            dnq = dnq_of.pop(qc)
            rcq = rcp.tile([2 * NPAIR, QC], f32, tag="rc", name=f"rc{qc}")
            nc.vector.reciprocal(rcq[:], dnq[:])
            rcd = dram.tile([NPAIR * 2, QC], f32, tag="rcd", name=f"rcd{qc}")
            nc.sync.dma_start(rcd[:], rcq[:])
            for j in range(NPAIR):
                bc_t = bcp.tile([P, QC], f32, tag="bc")
                nc.sync.dma_start(
                    bc_t[0:64, :],
                    rcd[2 * j:2 * j + 1, :].to_broadcast([64, QC]),
                )
                nc.sync.dma_start(
                    bc_t[64:P, :],
                    rcd[2 * j + 1:2 * j + 2, :].to_broadcast([64, QC]),
                )
                nc.vector.tensor_tensor(
                    attn[j][:, qc * QC:(qc + 1) * QC],
                    attn[j][:, qc * QC:(qc + 1) * QC],
                    bc_t[:],
                    MULT,
                )

        def o_proj_q(qc):
            if qc == 0:
                nc.sync.dma_start(wo[:], WoT[:].rearrange("(c p) d -> p c d", p=P))
            if qc >= NQC - 2:
                c0, cw = qc * QC, QC       # last two qc: drain per-chunk
            elif qc % 2 == 1:
                c0, cw = (qc // 2) * 512, 512
            else:
                return
            for dt in range(8):
                ps = acc.tile([P, 512], f32, tag="acc", name=f"op{qc}_{dt}")
                for ct in range(NPAIR):
                    nc.tensor.matmul(
                        out=ps[:, 0:cw],
                        lhsT=wo[:, ct, dt * P:(dt + 1) * P],
                        rhs=attn[ct][:, c0:c0 + cw],
                        start=(ct == 0), stop=(ct == NPAIR - 1),
                    )
                o_t = ost.tile([P, 512], f32, tag="ost")
                nc.vector.tensor_scalar_add(
                    o_t[:, 0:cw], ps[:, 0:cw], bo_sb[:, dt:dt + 1]
                )
                nc.sync.dma_start(
                    outT[dt * P:(dt + 1) * P, c0:c0 + cw], o_t[:, 0:cw]
                )

        # --- pipelined emission: PV lags QK by 3 units. Unit order runs
        # pairs {0,1} of qc0/qc1 first so PV can start after only the first
        # V half-pass; pairs {2,3} follow once the second half lands. Q sc1-3
        # and the V passes fill PE slack under the ACT-bound stream. ---
        units = (
            [(0, 0), (0, 1), (1, 0), (1, 1), (0, 2), (0, 3), (1, 2), (1, 3)]
            + [(qc, j) for qc in range(2, NQC) for j in range(NPAIR)]
        )
        LAG = 3
        q_proj_sc(0)
        fills = {
            3: lambda: v_proj_half(0),   # before pv(qc0,j0) at slot 3
            4: lambda: q_proj_sc(1),
            6: lambda: v_proj_half(1),   # before pv(qc0,j2) at slot 7
            9: lambda: q_proj_sc(2),
            13: lambda: q_proj_sc(3),
        }
        n_units = len(units)
        pv_next = 0
        for u in range(n_units):
            if u in fills:
                fills[u]()
            att_qk(*units[u])
            # taper the PV lag near the end so the pipeline drains early
            lag = LAG if u < n_units - 5 else max(1, n_units - 1 - u)
            while pv_next <= u - lag:
                att_pv(*units[pv_next])
                pv_next += 1
        while pv_next < n_units:
            att_pv(*units[pv_next])
            pv_next += 1

    if not nc.is_finalized():
        nc.finalize()
    return nc


def get_nc():
    if "nc" not in _CACHE:
        _CACHE["nc"] = _build_nc()
    return _CACHE["nc"]


def make_in_maps(inputs):
    f16 = np.float16
    import ml_dtypes

    bf16 = ml_dtypes.bfloat16
    q = np.asarray(inputs["query"], np.float32)
    k = np.asarray(inputs["key"], np.float32)
    v = np.asarray(inputs["value"], np.float32)
    Wq = np.asarray(inputs["Wq"], np.float32) * SCALE  # fold softmax scale
    Wk = np.asarray(inputs["Wk"], np.float32)
    Wv = np.asarray(inputs["Wv"], np.float32)
    Wo = np.asarray(inputs["Wo"], np.float32)
    bq = np.asarray(inputs["bq"], np.float32) * SCALE
    bk = np.asarray(inputs["bk"], np.float32)
    bv = np.asarray(inputs["bv"], np.float32)
    bo_half = np.asarray(inputs["bo"], np.float32) * 0.5

    qT = [np.ascontiguousarray(q[b].T.astype(f16)) for b in range(B)]
    kTl = [np.ascontiguousarray(k[b].T.astype(f16)) for b in range(B)]
    vT = [np.ascontiguousarray(v[b].T.astype(f16)) for b in range(B)]
    WqTs = [np.ascontiguousarray(Wq.T[:, i * DL:(i + 1) * DL].astype(f16)) for i in range(2)]
    WkTs = [np.ascontiguousarray(Wk.T[:, i * DL:(i + 1) * DL].astype(f16)) for i in range(2)]
    WvTs = [np.ascontiguousarray(Wv.T[:, i * DL:(i + 1) * DL].astype(f16)) for i in range(2)]
    WoTs = [np.ascontiguousarray(Wo.T[i * DL:(i + 1) * DL, :].astype(bf16)) for i in range(2)]
    bqs = [np.ascontiguousarray(bq[i * DL:(i + 1) * DL]) for i in range(2)]
    bks = [np.ascontiguousarray(bk[i * DL:(i + 1) * DL]) for i in range(2)]
    bvs = [np.ascontiguousarray(bv[i * DL:(i + 1) * DL]).reshape(1, DL) for i in range(2)]

    in_maps = []
    for c in range(8):
        b, hh = c // 2, c % 2
        in_maps.append({
            "queryT": qT[b], "keyT": kTl[b], "valueT": vT[b],
            "WqT": WqTs[hh], "WkT": WkTs[hh], "WvT": WvTs[hh], "WoT": WoTs[hh],
            "bq": bqs[hh], "bk": bks[hh], "bv": bvs[hh], "bo_half": bo_half,
        })
    return in_maps


def assemble(results):
    out = np.empty((B, S, D), np.float32)
    for b in range(B):
        out[b] = (results[2 * b]["outT"] + results[2 * b + 1]["outT"]).T
    return out


def _numpy_fallback(inputs):
    q = np.asarray(inputs["query"], np.float64)
    k = np.asarray(inputs["key"], np.float64)
    v = np.asarray(inputs["value"], np.float64)
    Wq, bq = np.asarray(inputs["Wq"], np.float64), np.asarray(inputs["bq"], np.float64)
    Wk, bk = np.asarray(inputs["Wk"], np.float64), np.asarray(inputs["bk"], np.float64)
    Wv, bv = np.asarray(inputs["Wv"], np.float64), np.asarray(inputs["bv"], np.float64)
    Wo, bo = np.asarray(inputs["Wo"], np.float64), np.asarray(inputs["bo"], np.float64)
    qp = (q @ Wq.T + bq).reshape(B, S, H, DH).transpose(0, 2, 1, 3)
    kp = (k @ Wk.T + bk).reshape(B, S, H, DH).transpose(0, 2, 1, 3)
    vp = (v @ Wv.T + bv).reshape(B, S, H, DH).transpose(0, 2, 1, 3)
    e = np.einsum("bhqd,bhkd->bhqk", qp, kp) * SCALE
    mask = np.asarray(inputs["mask"])
    kpm = np.asarray(inputs["key_padding_mask"])
    e = np.where(mask == 0, -np.inf, e)
    e = np.where(kpm[:, None, None, :] == 0, -np.inf, e)
    e -= e.max(axis=-1, keepdims=True)
    p = np.exp(e)
    p /= p.sum(axis=-1, keepdims=True)
    o = np.einsum("bhqk,bhkd->bhqd", p, vp).transpose(0, 2, 1, 3).reshape(B, S, D)
    return (o @ Wo.T + bo).astype(np.float32)


def kernel(**inputs):
    mask = np.asarray(inputs["mask"])
    kpm = np.asarray(inputs["key_padding_mask"])
    if not (mask.all() and kpm.all()):
        return _numpy_fallback(inputs)
    from concourse.bass_utils import run_bass_kernel_spmd

    nc = get_nc()
    in_maps = make_in_maps(inputs)
    res = run_bass_kernel_spmd(nc, in_maps, list(range(8)))
    return assemble(res.results)


# revision 28
# speedup vs baseline: 1.0513x; 1.0293x over previous
"""Trainium2 Bass kernel for CustomAttention (B=4, S=2048, D=1024, H=16).

Sharding: 8 cores = 4 batches x 2 head-halves (8 heads each). Each core
computes Q/K/V projections for its 512 head-dims, attention for its 8 heads
over all 2048 queries, and a partial out-projection (contraction over its 512
dims). Host sums the two partial outputs per batch; bo/2 is added on each core
so the host sum carries the full bias.

Performance structure:
  - All matmul operands 16-bit (fp16 where the value range allows, bf16 where
    exp magnitudes flow: pt, v_pad, unnormalized attention). Full-rate PE.
  - softmax scale folded into Wq/bq host-side.
  - K^T/Q^T/attention SBUF-resident; inputs staged in [128,512] chunks.
  - Pipelined emission: K proj -> Q proj (pair-major, so attention starts as
    soon as pair 0's Q lands) -> V proj -> attention (qc-outer, pair-inner)
    with per-qc deferred normalization and out-projection chunks inlined every
    other qc. ScalarE exp is the critical engine; everything else hides under.
  - exp in {6,6,4}-kti segments (1536-elem calls amortize ACT fixed overhead,
    3 PSUM banks each, double buffered). PV for both heads accumulates into
    ONE PSUM bank ([65, 2, 256]; sequential per-head chains), ones-column of
    v_pad gives the softmax denominators; one batched reciprocal per qc.
  - mask / key_padding_mask are all-ones for this problem's inputs => identity;
    a numpy fallback handles the (never-hit) general case.
"""

import math

import numpy as np

B, S, D = 4, 2048, 1024
H, DH = 16, 64       # global heads
HL = 8               # local heads per core
P = 128
NPAIR = HL // 2      # 4 local head pairs
NKT = S // P         # 16 key tiles
QC = 256             # query chunk for attention
NQC = S // QC        # 8
DL = 512             # local projection width (8 heads x 64)
SCALE = math.log(D) / math.sqrt(DH)
SEGS = [(0, 6), (6, 6), (12, 4)]  # kti segments for QK/exp

_CACHE = {}


def _build_nc():
    import concourse.bass as bass
    import concourse.bacc as bacc
    import concourse.mybir as mybir
    import concourse.tile as tile
    from contextlib import ExitStack

    f32 = mybir.dt.float32
    f16 = mybir.dt.float16
    bf16 = mybir.dt.bfloat16
    EXP = mybir.ActivationFunctionType.Exp
    ADD = mybir.AluOpType.add
    MULT = mybir.AluOpType.mult

    nc = bacc.Bacc("TRN2", target_bir_lowering=False, debug=False, num_devices=8)

    queryT = nc.declare_dram_parameter("queryT", [D, S], f16, isOutput=False)
    keyT = nc.declare_dram_parameter("keyT", [D, S], f16, isOutput=False)
    valueT = nc.declare_dram_parameter("valueT", [D, S], f16, isOutput=False)
    WqT = nc.declare_dram_parameter("WqT", [D, DL], f16, isOutput=False)
    WkT = nc.declare_dram_parameter("WkT", [D, DL], f16, isOutput=False)
    WvT = nc.declare_dram_parameter("WvT", [D, DL], f16, isOutput=False)
    WoT = nc.declare_dram_parameter("WoT", [DL, D], bf16, isOutput=False)
    bq_d = nc.declare_dram_parameter("bq", [DL], f32, isOutput=False)
    bk_d = nc.declare_dram_parameter("bk", [DL], f32, isOutput=False)
    bv_d = nc.declare_dram_parameter("bv", [1, DL], f32, isOutput=False)
    bo_d = nc.declare_dram_parameter("bo_half", [D], f32, isOutput=False)
    outT = nc.declare_dram_parameter("outT", [D, S], f32, isOutput=True)

    with ExitStack() as ctx:
        tc = ctx.enter_context(tile.TileContext(nc))
        persist = ctx.enter_context(tc.tile_pool(name="persist", bufs=1))
        wpool = ctx.enter_context(tc.tile_pool(name="wpool", bufs=1))
        in512 = ctx.enter_context(tc.tile_pool(name="in512", bufs=24))
        ptp = ctx.enter_context(tc.tile_pool(name="ptp", bufs=8))
        bcp = ctx.enter_context(tc.tile_pool(name="bcp", bufs=4))
        stgp = ctx.enter_context(tc.tile_pool(name="stgp", bufs=2))
        ost = ctx.enter_context(tc.tile_pool(name="ost", bufs=2))
        dnp = ctx.enter_context(tc.tile_pool(name="dnp", bufs=2))
        rcp = ctx.enter_context(tc.tile_pool(name="rcp", bufs=2))
        et = ctx.enter_context(tc.tile_pool(name="et", bufs=2, space="PSUM"))
        pvp = ctx.enter_context(tc.tile_pool(name="pvp", bufs=1, space="PSUM"))
        acc = ctx.enter_context(tc.tile_pool(name="acc", bufs=1, space="PSUM"))
        dram = ctx.enter_context(tc.tile_pool(name="dram", bufs=2, space="DRAM"))

        kT = [
            persist.tile([P, S], f16, tag=f"kT{j}", name=f"kT{j}")
            for j in range(NPAIR)
        ]
        qt = [
            persist.tile([P, S], f16, tag=f"qt{j}", name=f"qt{j}")
            for j in range(NPAIR)
        ]
        attn = [
            persist.tile([P, S], bf16, tag=f"at{j}", name=f"at{j}")
            for j in range(NPAIR)
        ]
        v_pad = persist.tile([P, NKT, HL, DH + 1], bf16, tag="v_pad")
        bq_sb = persist.tile([P, NPAIR], f32, tag="bq")
        bk_sb = persist.tile([P, NPAIR], f32, tag="bk")
        bo_sb = persist.tile([P, 8], f32, tag="bo")
        bv_bc = persist.tile([P, DL], f32, tag="bv_bc")

        # --- setup ---
        nc.sync.dma_start(bq_sb[:], bq_d.rearrange("(o p) -> p o", p=P))
        nc.sync.dma_start(bk_sb[:], bk_d.rearrange("(o p) -> p o", p=P))
        nc.sync.dma_start(bo_sb[:], bo_d.rearrange("(o p) -> p o", p=P))
        nc.sync.dma_start(bv_bc[:], bv_d[:].to_broadcast([P, DL]))
        nc.vector.memset(v_pad[:], 1.0)

        # weights DMA'd lazily at first use so k-input chunks go first
        wk = wpool.tile([P, 8, DL], f16, tag="w", name="wk")
        nc.sync.dma_start(wk[:], WkT[:].rearrange("(k p) c -> p k c", p=P))
        wq = wpool.tile([P, 8, DL], f16, tag="w2", name="wq")
        wv = wpool.tile([P, 8, DL], f16, tag="w3", name="wv")
        wo = wpool.tile([P, NPAIR, D], bf16, tag="w4", name="wo")

        def chunk(srcT, kt, sc, name):
            t = in512.tile([P, 512], f16, tag="in", name=name)
            nc.sync.dma_start(
                t[:], srcT[kt * P:(kt + 1) * P, sc * 512:(sc + 1) * 512]
            )
            return t

        def proj_psum(i):
            # alternate between the two 1-bank pools for 2-deep pipelining
            pool = acc if i % 2 == 0 else pvp
            return pool.tile([P, 512], f32, tag="acc" if i % 2 == 0 else "pv", name=f"pp{i}")

        # --- K projection: kT[j][p(2h x 64dh), seq], sc-major ---
        pi = 0
        for sc in range(4):
            kc = [chunk(keyT, kt, sc, f"kc{kt}_{sc}") for kt in range(8)]
            for j in range(NPAIR):
                ps = proj_psum(pi); pi += 1
                for kt in range(8):
                    nc.tensor.matmul(
                        out=ps[:],
                        lhsT=wk[:, kt, j * P:(j + 1) * P],
                        rhs=kc[kt][:],
                        start=(kt == 0), stop=(kt == 7),
                    )
                nc.vector.tensor_scalar_add(
                    kT[j][:, sc * 512:(sc + 1) * 512], ps[:], bk_sb[:, j:j + 1]
                )

        # --- Q projection emitter (one sc chunk of all pairs at a time) ---
        def q_proj_sc(sc):
            nonlocal pi
            if sc == 0:
                nc.sync.dma_start(wq[:], WqT[:].rearrange("(k p) c -> p k c", p=P))
            qc_ = [chunk(queryT, kt, sc, f"qc{kt}_{sc}") for kt in range(8)]
            for j in range(NPAIR):
                ps = proj_psum(pi); pi += 1
                for kt in range(8):
                    nc.tensor.matmul(
                        out=ps[:],
                        lhsT=wq[:, kt, j * P:(j + 1) * P],
                        rhs=qc_[kt][:],
                        start=(kt == 0), stop=(kt == 7),
                    )
                nc.vector.tensor_scalar_add(
                    qt[j][:, sc * 512:(sc + 1) * 512], ps[:], bq_sb[:, j:j + 1]
                )

        # --- V projection emitter: one pair-group (2 pairs = 256 dims) pass.
        # Split so PV of early pairs can start before all of V is projected. ---
        def v_proj_half(g):
            nonlocal pi
            if g == 0:
                nc.sync.dma_start(wv[:], WvT[:].rearrange("(k p) c -> p k c", p=P))
            for stg4 in range(4):
                vc = [
                    chunk(valueT, kt, stg4, f"vc{g}_{kt}_{stg4}")
                    for kt in range(8)
                ]
                for stl in range(4):
                    st = stg4 * 4 + stl
                    ps = proj_psum(pi); pi += 1
                    for kt in range(8):
                        nc.tensor.matmul(
                            out=ps[:, 0:256],
                            lhsT=vc[kt][:, stl * P:(stl + 1) * P],
                            rhs=wv[:, kt, g * 256:(g + 1) * 256],
                            start=(kt == 0), stop=(kt == 7),
                        )
                    nc.vector.tensor_tensor(
                        v_pad[:, st, g * 4:(g + 1) * 4, 0:DH],
                        ps[:, 0:256].rearrange("p (h d) -> p h d", h=4),
                        bv_bc[:, g * 256:(g + 1) * 256].rearrange(
                            "p (h d) -> p h d", h=4
                        ),
                        ADD,
                    )

        # --- attention emitters: QK+exp and (staggered) PV+normalization ---
        pt_of = {}
        dnq_of = {}

        def att_qk(qc, j):
            pt = [
                ptp.tile([P, NKT, QC], bf16, tag="pt", name=f"pt{qc}_{j}_{h}")
                for h in range(2)
            ]
            pt_of[(qc, j)] = pt
            for sb, sl in SEGS:
                ett = [
                    et.tile([P, 6, QC], f32, tag="et", name=f"et{h}")
                    for h in range(2)
                ]
                for t_i in range(sl):
                    kti = sb + t_i
                    for h in range(2):
                        nc.tensor.matmul(
                            out=ett[h][:, t_i, :],
                            lhsT=kT[j][h * 64:(h + 1) * 64, kti * P:(kti + 1) * P],
                            rhs=qt[j][h * 64:(h + 1) * 64, qc * QC:(qc + 1) * QC],
                            start=True, stop=True,
                            tile_position=(h * 64, 0),
                        )
                for h in range(2):
                    nc.scalar.activation(
                        pt[h][:, sb:sb + sl, :], ett[h][:, 0:sl, :], EXP
                    )

        pv_done = {qc: 0 for qc in range(NQC)}

        def att_pv(qc, j):
            if qc not in dnq_of:
                dnq_of[qc] = dnp.tile(
                    [2 * NPAIR, QC], f32, tag="dn", name=f"dn{qc}"
                )
            dnq = dnq_of[qc]
            pt = pt_of.pop((qc, j))
            pvt = pvp.tile([DH + 1, 2, QC], f32, tag="pv", name=f"pv{qc}_{j}")
            for h in range(2):
                for kti in range(NKT):
                    nc.tensor.matmul(
                        out=pvt[0:DH + 1, h, :],
                        lhsT=v_pad[:, kti, 2 * j + h, 0:DH + 1],
                        rhs=pt[h][:, kti, :],
                        start=(kti == 0), stop=(kti == NKT - 1),
                    )
            stg = stgp.tile([P, 2, QC], f32, tag="dnst")
            for h in range(2):
                nc.vector.tensor_copy(
                    out=attn[j][h * 64:(h + 1) * 64, qc * QC:(qc + 1) * QC],
                    in_=pvt[0:DH, h, :],
                )
                nc.vector.tensor_copy(
                    out=stg[64:65, h, :], in_=pvt[DH:DH + 1, h, :]
                )
            nc.sync.dma_start(dnq[2 * j:2 * j + 2, :], stg[64:65, :, :])
            pv_done[qc] += 1
            if pv_done[qc] == NPAIR:
                norm(qc)
                o_proj_q(qc)

        def norm(qc):
            dnq = dnq_of.pop(qc)
            rcq = rcp.tile([2 * NPAIR, QC], f32, tag="rc", name=f"rc{qc}")
            nc.vector.reciprocal(rcq[:], dnq[:])
            rcd = dram.tile([NPAIR * 2, QC], f32, tag="rcd", name=f"rcd{qc}")
            nc.sync.dma_start(rcd[:], rcq[:])
            for j in range(NPAIR):
                bc_t = bcp.tile([P, QC], f32, tag="bc")
                nc.sync.dma_start(
                    bc_t[0:64, :],
                    rcd[2 * j:2 * j + 1, :].to_broadcast([64, QC]),
                )
                nc.sync.dma_start(
                    bc_t[64:P, :],
                    rcd[2 * j + 1:2 * j + 2, :].to_broadcast([64, QC]),
                )
                nc.vector.tensor_tensor(
                    attn[j][:, qc * QC:(qc + 1) * QC],
                    attn[j][:, qc * QC:(qc + 1) * QC],
                    bc_t[:],
                    MULT,
                )

        def o_proj_q(qc):
            if qc == 0:
                nc.sync.dma_start(wo[:], WoT[:].rearrange("(c p) d -> p c d", p=P))
            if qc >= NQC - 2:
                c0, cw = qc * QC, QC       # last two qc: drain per-chunk
            elif qc % 2 == 1:
                c0, cw = (qc // 2) * 512, 512
            else:
                return
            for dt in range(8):
                ps = acc.tile([P, 512], f32, tag="acc", name=f"op{qc}_{dt}")
                for ct in range(NPAIR):
                    nc.tensor.matmul(
                        out=ps[:, 0:cw],
                        lhsT=wo[:, ct, dt * P:(dt + 1) * P],
                        rhs=attn[ct][:, c0:c0 + cw],
                        start=(ct == 0), stop=(ct == NPAIR - 1),
                    )
                o_t = ost.tile([P, 512], f32, tag="ost")
                nc.vector.tensor_scalar_add(
                    o_t[:, 0:cw], ps[:, 0:cw], bo_sb[:, dt:dt + 1]
                )
                nc.sync.dma_start(
                    outT[dt * P:(dt + 1) * P, c0:c0 + cw], o_t[:, 0:cw]
                )

        # --- pipelined emission: PV lags QK by 3 units. Unit order runs
        # pairs {0,1} of qc0/qc1 first so PV can start after only the first
        # V half-pass; pairs {2,3} follow once the second half lands. Q sc1-3
        # and the V passes fill PE slack under the ACT-bound stream. ---
        units = (
            [(0, 0), (0, 1), (1, 0), (1, 1), (0, 2), (0, 3), (1, 2), (1, 3)]
            + [(qc, j) for qc in range(2, NQC) for j in range(NPAIR)]
        )
        LAG = 3
        q_proj_sc(0)
        fills = {
            3: lambda: v_proj_half(0),   # before pv(qc0,j0) at slot 3
            4: lambda: q_proj_sc(1),
            6: lambda: v_proj_half(1),   # before pv(qc0,j2) at slot 7
            12: lambda: q_proj_sc(2),
            20: lambda: q_proj_sc(3),
        }
        n_units = len(units)
        pv_next = 0
        for u in range(n_units):
            if u in fills:
                fills[u]()
            att_qk(*units[u])
            # taper the PV lag near the end so the pipeline drains early
            lag = LAG if u < n_units - 5 else max(1, n_units - 1 - u)
            while pv_next <= u - lag:
                att_pv(*units[pv_next])
                pv_next += 1
        while pv_next < n_units:
            att_pv(*units[pv_next])
            pv_next += 1

    if not nc.is_finalized():
        nc.finalize()
    return nc


def get_nc():
    if "nc" not in _CACHE:
        _CACHE["nc"] = _build_nc()
    return _CACHE["nc"]


def make_in_maps(inputs):
    f16 = np.float16
    import ml_dtypes

    bf16 = ml_dtypes.bfloat16
    q = np.asarray(inputs["query"], np.float32)
    k = np.asarray(inputs["key"], np.float32)
    v = np.asarray(inputs["value"], np.float32)
    Wq = np.asarray(inputs["Wq"], np.float32) * SCALE  # fold softmax scale
    Wk = np.asarray(inputs["Wk"], np.float32)
    Wv = np.asarray(inputs["Wv"], np.float32)
    Wo = np.asarray(inputs["Wo"], np.float32)
    bq = np.asarray(inputs["bq"], np.float32) * SCALE
    bk = np.asarray(inputs["bk"], np.float32)
    bv = np.asarray(inputs["bv"], np.float32)
    bo_half = np.asarray(inputs["bo"], np.float32) * 0.5

    qT = [np.ascontiguousarray(q[b].T.astype(f16)) for b in range(B)]
    kTl = [np.ascontiguousarray(k[b].T.astype(f16)) for b in range(B)]
    vT = [np.ascontiguousarray(v[b].T.astype(f16)) for b in range(B)]
    WqTs = [np.ascontiguousarray(Wq.T[:, i * DL:(i + 1) * DL].astype(f16)) for i in range(2)]
    WkTs = [np.ascontiguousarray(Wk.T[:, i * DL:(i + 1) * DL].astype(f16)) for i in range(2)]
    WvTs = [np.ascontiguousarray(Wv.T[:, i * DL:(i + 1) * DL].astype(f16)) for i in range(2)]
    WoTs = [np.ascontiguousarray(Wo.T[i * DL:(i + 1) * DL, :].astype(bf16)) for i in range(2)]
    bqs = [np.ascontiguousarray(bq[i * DL:(i + 1) * DL]) for i in range(2)]
    bks = [np.ascontiguousarray(bk[i * DL:(i + 1) * DL]) for i in range(2)]
    bvs = [np.ascontiguousarray(bv[i * DL:(i + 1) * DL]).reshape(1, DL) for i in range(2)]

    in_maps = []
    for c in range(8):
        b, hh = c // 2, c % 2
        in_maps.append({
            "queryT": qT[b], "keyT": kTl[b], "valueT": vT[b],
            "WqT": WqTs[hh], "WkT": WkTs[hh], "WvT": WvTs[hh], "WoT": WoTs[hh],
            "bq": bqs[hh], "bk": bks[hh], "bv": bvs[hh], "bo_half": bo_half,
        })
    return in_maps


def assemble(results):
    out = np.empty((B, S, D), np.float32)
    for b in range(B):
        out[b] = (results[2 * b]["outT"] + results[2 * b + 1]["outT"]).T
    return out


def _numpy_fallback(inputs):
    q = np.asarray(inputs["query"], np.float64)
    k = np.asarray(inputs["key"], np.float64)
    v = np.asarray(inputs["value"], np.float64)
    Wq, bq = np.asarray(inputs["Wq"], np.float64), np.asarray(inputs["bq"], np.float64)
    Wk, bk = np.asarray(inputs["Wk"], np.float64), np.asarray(inputs["bk"], np.float64)
    Wv, bv = np.asarray(inputs["Wv"], np.float64), np.asarray(inputs["bv"], np.float64)
    Wo, bo = np.asarray(inputs["Wo"], np.float64), np.asarray(inputs["bo"], np.float64)
    qp = (q @ Wq.T + bq).reshape(B, S, H, DH).transpose(0, 2, 1, 3)
    kp = (k @ Wk.T + bk).reshape(B, S, H, DH).transpose(0, 2, 1, 3)
    vp = (v @ Wv.T + bv).reshape(B, S, H, DH).transpose(0, 2, 1, 3)
    e = np.einsum("bhqd,bhkd->bhqk", qp, kp) * SCALE
    mask = np.asarray(inputs["mask"])
    kpm = np.asarray(inputs["key_padding_mask"])
    e = np.where(mask == 0, -np.inf, e)
    e = np.where(kpm[:, None, None, :] == 0, -np.inf, e)
    e -= e.max(axis=-1, keepdims=True)
    p = np.exp(e)
    p /= p.sum(axis=-1, keepdims=True)
    o = np.einsum("bhqk,bhkd->bhqd", p, vp).transpose(0, 2, 1, 3).reshape(B, S, D)
    return (o @ Wo.T + bo).astype(np.float32)


def kernel(**inputs):
    mask = np.asarray(inputs["mask"])
    kpm = np.asarray(inputs["key_padding_mask"])
    if not (mask.all() and kpm.all()):
        return _numpy_fallback(inputs)
    from concourse.bass_utils import run_bass_kernel_spmd

    nc = get_nc()
    in_maps = make_in_maps(inputs)
    res = run_bass_kernel_spmd(nc, in_maps, list(range(8)))
    return assemble(res.results)


# revision 29
# speedup vs baseline: 1.0607x; 1.0090x over previous
"""Trainium2 Bass kernel for CustomAttention (B=4, S=2048, D=1024, H=16).

Sharding: 8 cores = 4 batches x 2 head-halves (8 heads each). Each core
computes Q/K/V projections for its 512 head-dims, attention for its 8 heads
over all 2048 queries, and a partial out-projection (contraction over its 512
dims). Host sums the two partial outputs per batch; bo/2 is added on each core
so the host sum carries the full bias.

Performance structure:
  - All matmul operands 16-bit (fp16 where the value range allows, bf16 where
    exp magnitudes flow: pt, v_pad, unnormalized attention). Full-rate PE.
  - softmax scale folded into Wq/bq host-side.
  - K^T/Q^T/attention SBUF-resident; inputs staged in [128,512] chunks.
  - Pipelined emission: K proj -> Q proj (pair-major, so attention starts as
    soon as pair 0's Q lands) -> V proj -> attention (qc-outer, pair-inner)
    with per-qc deferred normalization and out-projection chunks inlined every
    other qc. ScalarE exp is the critical engine; everything else hides under.
  - exp in {6,6,4}-kti segments (1536-elem calls amortize ACT fixed overhead,
    3 PSUM banks each, double buffered). PV for both heads accumulates into
    ONE PSUM bank ([65, 2, 256]; sequential per-head chains), ones-column of
    v_pad gives the softmax denominators; one batched reciprocal per qc.
  - mask / key_padding_mask are all-ones for this problem's inputs => identity;
    a numpy fallback handles the (never-hit) general case.
"""

import math

import numpy as np

B, S, D = 4, 2048, 1024
H, DH = 16, 64       # global heads
HL = 8               # local heads per core
P = 128
NPAIR = HL // 2      # 4 local head pairs
NKT = S // P         # 16 key tiles
QC = 256             # query chunk for attention
NQC = S // QC        # 8
DL = 512             # local projection width (8 heads x 64)
SCALE = math.log(D) / math.sqrt(DH)
SEGS = [(0, 6), (6, 6), (12, 4)]  # kti segments for QK/exp

_CACHE = {}


def _build_nc():
    import concourse.bass as bass
    import concourse.bacc as bacc
    import concourse.mybir as mybir
    import concourse.tile as tile
    from contextlib import ExitStack

    f32 = mybir.dt.float32
    f16 = mybir.dt.float16
    bf16 = mybir.dt.bfloat16
    EXP = mybir.ActivationFunctionType.Exp
    ADD = mybir.AluOpType.add
    MULT = mybir.AluOpType.mult

    nc = bacc.Bacc("TRN2", target_bir_lowering=False, debug=False, num_devices=8)

    queryT = nc.declare_dram_parameter("queryT", [D, S], f16, isOutput=False)
    keyT = nc.declare_dram_parameter("keyT", [D, S], f16, isOutput=False)
    valueT = nc.declare_dram_parameter("valueT", [D, S], f16, isOutput=False)
    WqT = nc.declare_dram_parameter("WqT", [D, DL], f16, isOutput=False)
    WkT = nc.declare_dram_parameter("WkT", [D, DL], f16, isOutput=False)
    WvT = nc.declare_dram_parameter("WvT", [D, DL], f16, isOutput=False)
    WoT = nc.declare_dram_parameter("WoT", [DL, D], bf16, isOutput=False)
    bq_d = nc.declare_dram_parameter("bq", [DL], f32, isOutput=False)
    bk_d = nc.declare_dram_parameter("bk", [DL], f32, isOutput=False)
    bv_d = nc.declare_dram_parameter("bv", [1, DL], f32, isOutput=False)
    bo_d = nc.declare_dram_parameter("bo_half", [D], f32, isOutput=False)
    outT = nc.declare_dram_parameter("outT", [D, S], f32, isOutput=True)

    with ExitStack() as ctx:
        tc = ctx.enter_context(tile.TileContext(nc))
        persist = ctx.enter_context(tc.tile_pool(name="persist", bufs=1))
        wpool = ctx.enter_context(tc.tile_pool(name="wpool", bufs=1))
        in512 = ctx.enter_context(tc.tile_pool(name="in512", bufs=24))
        ptp = ctx.enter_context(tc.tile_pool(name="ptp", bufs=8))
        bcp = ctx.enter_context(tc.tile_pool(name="bcp", bufs=4))
        stgp = ctx.enter_context(tc.tile_pool(name="stgp", bufs=2))
        ost = ctx.enter_context(tc.tile_pool(name="ost", bufs=2))
        dnp = ctx.enter_context(tc.tile_pool(name="dnp", bufs=2))
        rcp = ctx.enter_context(tc.tile_pool(name="rcp", bufs=2))
        et = ctx.enter_context(tc.tile_pool(name="et", bufs=2, space="PSUM"))
        pvp = ctx.enter_context(tc.tile_pool(name="pvp", bufs=1, space="PSUM"))
        acc = ctx.enter_context(tc.tile_pool(name="acc", bufs=1, space="PSUM"))
        dram = ctx.enter_context(tc.tile_pool(name="dram", bufs=2, space="DRAM"))

        kT = [
            persist.tile([P, S], f16, tag=f"kT{j}", name=f"kT{j}")
            for j in range(NPAIR)
        ]
        qt = [
            persist.tile([P, S], f16, tag=f"qt{j}", name=f"qt{j}")
            for j in range(NPAIR)
        ]
        attn = [
            persist.tile([P, S], bf16, tag=f"at{j}", name=f"at{j}")
            for j in range(NPAIR)
        ]
        v_pad = persist.tile([P, NKT, HL, DH + 1], bf16, tag="v_pad")
        bq_sb = persist.tile([P, NPAIR], f32, tag="bq")
        bk_sb = persist.tile([P, NPAIR], f32, tag="bk")
        bo_sb = persist.tile([P, 8], f32, tag="bo")
        bv_bc = persist.tile([P, DL], f32, tag="bv_bc")

        # --- setup ---
        nc.sync.dma_start(bq_sb[:], bq_d.rearrange("(o p) -> p o", p=P))
        nc.sync.dma_start(bk_sb[:], bk_d.rearrange("(o p) -> p o", p=P))
        nc.sync.dma_start(bo_sb[:], bo_d.rearrange("(o p) -> p o", p=P))
        nc.sync.dma_start(bv_bc[:], bv_d[:].to_broadcast([P, DL]))
        nc.vector.memset(v_pad[:], 1.0)

        # weights DMA'd lazily at first use so k-input chunks go first
        wk = wpool.tile([P, 8, DL], f16, tag="w", name="wk")
        nc.sync.dma_start(wk[:], WkT[:].rearrange("(k p) c -> p k c", p=P))
        wq = wpool.tile([P, 8, DL], f16, tag="w2", name="wq")
        wv = wpool.tile([P, 8, DL], f16, tag="w3", name="wv")
        wo = wpool.tile([P, NPAIR, D], bf16, tag="w4", name="wo")

        def chunk(srcT, kt, sc, name):
            t = in512.tile([P, 512], f16, tag="in", name=name)
            nc.sync.dma_start(
                t[:], srcT[kt * P:(kt + 1) * P, sc * 512:(sc + 1) * 512]
            )
            return t

        def proj_psum(i):
            # alternate between the two 1-bank pools for 2-deep pipelining
            pool = acc if i % 2 == 0 else pvp
            return pool.tile([P, 512], f32, tag="acc" if i % 2 == 0 else "pv", name=f"pp{i}")

        # --- K projection: kT[j][p(2h x 64dh), seq], sc-major ---
        pi = 0
        for sc in range(4):
            kc = [chunk(keyT, kt, sc, f"kc{kt}_{sc}") for kt in range(8)]
            for j in range(NPAIR):
                ps = proj_psum(pi); pi += 1
                for kt in range(8):
                    nc.tensor.matmul(
                        out=ps[:],
                        lhsT=wk[:, kt, j * P:(j + 1) * P],
                        rhs=kc[kt][:],
                        start=(kt == 0), stop=(kt == 7),
                    )
                nc.vector.tensor_scalar_add(
                    kT[j][:, sc * 512:(sc + 1) * 512], ps[:], bk_sb[:, j:j + 1]
                )

        # --- Q projection emitter (one sc chunk of all pairs at a time) ---
        def q_proj_sc(sc):
            nonlocal pi
            if sc == 0:
                nc.sync.dma_start(wq[:], WqT[:].rearrange("(k p) c -> p k c", p=P))
            qc_ = [chunk(queryT, kt, sc, f"qc{kt}_{sc}") for kt in range(8)]
            for j in range(NPAIR):
                ps = proj_psum(pi); pi += 1
                for kt in range(8):
                    nc.tensor.matmul(
                        out=ps[:],
                        lhsT=wq[:, kt, j * P:(j + 1) * P],
                        rhs=qc_[kt][:],
                        start=(kt == 0), stop=(kt == 7),
                    )
                nc.vector.tensor_scalar_add(
                    qt[j][:, sc * 512:(sc + 1) * 512], ps[:], bq_sb[:, j:j + 1]
                )

        # --- V projection emitter: one pair-group (2 pairs = 256 dims) pass.
        # Split so PV of early pairs can start before all of V is projected. ---
        def v_proj_half(g):
            nonlocal pi
            if g == 0:
                nc.sync.dma_start(wv[:], WvT[:].rearrange("(k p) c -> p k c", p=P))
            for stg4 in range(4):
                vc = [
                    chunk(valueT, kt, stg4, f"vc{g}_{kt}_{stg4}")
                    for kt in range(8)
                ]
                for stl in range(4):
                    st = stg4 * 4 + stl
                    ps = proj_psum(pi); pi += 1
                    for kt in range(8):
                        nc.tensor.matmul(
                            out=ps[:, 0:256],
                            lhsT=vc[kt][:, stl * P:(stl + 1) * P],
                            rhs=wv[:, kt, g * 256:(g + 1) * 256],
                            start=(kt == 0), stop=(kt == 7),
                        )
                    nc.vector.tensor_tensor(
                        v_pad[:, st, g * 4:(g + 1) * 4, 0:DH],
                        ps[:, 0:256].rearrange("p (h d) -> p h d", h=4),
                        bv_bc[:, g * 256:(g + 1) * 256].rearrange(
                            "p (h d) -> p h d", h=4
                        ),
                        ADD,
                    )

        # --- attention emitters: QK+exp and (staggered) PV+normalization ---
        pt_of = {}
        dnq_of = {}

        def att_qk(qc, j):
            pt = [
                ptp.tile([P, NKT, QC], bf16, tag="pt", name=f"pt{qc}_{j}_{h}")
                for h in range(2)
            ]
            pt_of[(qc, j)] = pt
            for sb, sl in SEGS:
                ett = [
                    et.tile([P, 6, QC], f32, tag="et", name=f"et{h}")
                    for h in range(2)
                ]
                for t_i in range(sl):
                    kti = sb + t_i
                    for h in range(2):
                        nc.tensor.matmul(
                            out=ett[h][:, t_i, :],
                            lhsT=kT[j][h * 64:(h + 1) * 64, kti * P:(kti + 1) * P],
                            rhs=qt[j][h * 64:(h + 1) * 64, qc * QC:(qc + 1) * QC],
                            start=True, stop=True,
                            tile_position=(h * 64, 0),
                        )
                for h in range(2):
                    nc.scalar.activation(
                        pt[h][:, sb:sb + sl, :], ett[h][:, 0:sl, :], EXP
                    )

        pv_done = {qc: 0 for qc in range(NQC)}

        def att_pv(qc, j):
            if qc not in dnq_of:
                dnq_of[qc] = dnp.tile(
                    [2 * NPAIR, QC], f32, tag="dn", name=f"dn{qc}"
                )
            dnq = dnq_of[qc]
            pt = pt_of.pop((qc, j))
            pvt = pvp.tile([DH + 1, 2, QC], f32, tag="pv", name=f"pv{qc}_{j}")
            for h in range(2):
                for kti in range(NKT):
                    nc.tensor.matmul(
                        out=pvt[0:DH + 1, h, :],
                        lhsT=v_pad[:, kti, 2 * j + h, 0:DH + 1],
                        rhs=pt[h][:, kti, :],
                        start=(kti == 0), stop=(kti == NKT - 1),
                    )
            stg = stgp.tile([P, 2, QC], f32, tag="dnst")
            for h in range(2):
                nc.vector.tensor_copy(
                    out=attn[j][h * 64:(h + 1) * 64, qc * QC:(qc + 1) * QC],
                    in_=pvt[0:DH, h, :],
                )
                nc.vector.tensor_copy(
                    out=stg[64:65, h, :], in_=pvt[DH:DH + 1, h, :]
                )
            nc.sync.dma_start(dnq[2 * j:2 * j + 2, :], stg[64:65, :, :])
            pv_done[qc] += 1
            if pv_done[qc] == NPAIR:
                norm(qc)
                o_proj_q(qc)

        def norm(qc):
            dnq = dnq_of.pop(qc)
            rcq = rcp.tile([2 * NPAIR, QC], f32, tag="rc", name=f"rc{qc}")
            nc.vector.reciprocal(rcq[:], dnq[:])
            rcd = dram.tile([NPAIR * 2, QC], f32, tag="rcd", name=f"rcd{qc}")
            nc.sync.dma_start(rcd[:], rcq[:])
            for j in range(NPAIR):
                bc_t = bcp.tile([P, QC], f32, tag="bc")
                nc.sync.dma_start(
                    bc_t[0:64, :],
                    rcd[2 * j:2 * j + 1, :].to_broadcast([64, QC]),
                )
                nc.sync.dma_start(
                    bc_t[64:P, :],
                    rcd[2 * j + 1:2 * j + 2, :].to_broadcast([64, QC]),
                )
                nc.vector.tensor_tensor(
                    attn[j][:, qc * QC:(qc + 1) * QC],
                    attn[j][:, qc * QC:(qc + 1) * QC],
                    bc_t[:],
                    MULT,
                )

        o_chains = []

        def o_chain(qc, c0, cw, dt):
            ps = acc.tile([P, 512], f32, tag="acc", name=f"op{qc}_{dt}")
            for ct in range(NPAIR):
                nc.tensor.matmul(
                    out=ps[:, 0:cw],
                    lhsT=wo[:, ct, dt * P:(dt + 1) * P],
                    rhs=attn[ct][:, c0:c0 + cw],
                    start=(ct == 0), stop=(ct == NPAIR - 1),
                )
            o_t = ost.tile([P, 512], f32, tag="ost")
            nc.vector.tensor_scalar_add(
                o_t[:, 0:cw], ps[:, 0:cw], bo_sb[:, dt:dt + 1]
            )
            nc.sync.dma_start(
                outT[dt * P:(dt + 1) * P, c0:c0 + cw], o_t[:, 0:cw]
            )

        def o_proj_q(qc):
            # queue the 8 output-projection chains; the main loop trickles
            # them between attention units to avoid head-of-line PE stalls
            if qc == 0:
                nc.sync.dma_start(wo[:], WoT[:].rearrange("(c p) d -> p c d", p=P))
            if qc >= NQC - 2:
                c0, cw = qc * QC, QC       # last two qc: drain per-chunk
            elif qc % 2 == 1:
                c0, cw = (qc // 2) * 512, 512
            else:
                return
            for dt in range(8):
                o_chains.append((qc, c0, cw, dt))

        # --- pipelined emission: PV lags QK by 3 units. Unit order runs
        # pairs {0,1} of qc0/qc1 first so PV can start after only the first
        # V half-pass; pairs {2,3} follow once the second half lands. Q sc1-3
        # and the V passes fill PE slack under the ACT-bound stream. ---
        units = (
            [(0, 0), (0, 1), (1, 0), (1, 1), (0, 2), (0, 3), (1, 2), (1, 3)]
            + [(qc, j) for qc in range(2, NQC) for j in range(NPAIR)]
        )
        LAG = 3
        q_proj_sc(0)
        fills = {
            3: lambda: v_proj_half(0),   # before pv(qc0,j0) at slot 3
            4: lambda: q_proj_sc(1),
            6: lambda: v_proj_half(1),   # before pv(qc0,j2) at slot 7
            12: lambda: q_proj_sc(2),
            20: lambda: q_proj_sc(3),
        }
        n_units = len(units)
        pv_next = 0
        for u in range(n_units):
            if u in fills:
                fills[u]()
            att_qk(*units[u])
            # taper the PV lag near the end so the pipeline drains early
            lag = LAG if u < n_units - 5 else max(1, n_units - 1 - u)
            while pv_next <= u - lag:
                att_pv(*units[pv_next])
                pv_next += 1
            for _ in range(2 if len(o_chains) > 8 else 1):
                if o_chains:
                    o_chain(*o_chains.pop(0))
        while pv_next < n_units:
            att_pv(*units[pv_next])
            pv_next += 1
            while o_chains:
                o_chain(*o_chains.pop(0))

    if not nc.is_finalized():
        nc.finalize()
    return nc


def get_nc():
    if "nc" not in _CACHE:
        _CACHE["nc"] = _build_nc()
    return _CACHE["nc"]


def make_in_maps(inputs):
    f16 = np.float16
    import ml_dtypes

    bf16 = ml_dtypes.bfloat16
    q = np.asarray(inputs["query"], np.float32)
    k = np.asarray(inputs["key"], np.float32)
    v = np.asarray(inputs["value"], np.float32)
    Wq = np.asarray(inputs["Wq"], np.float32) * SCALE  # fold softmax scale
    Wk = np.asarray(inputs["Wk"], np.float32)
    Wv = np.asarray(inputs["Wv"], np.float32)
    Wo = np.asarray(inputs["Wo"], np.float32)
    bq = np.asarray(inputs["bq"], np.float32) * SCALE
    bk = np.asarray(inputs["bk"], np.float32)
    bv = np.asarray(inputs["bv"], np.float32)
    bo_half = np.asarray(inputs["bo"], np.float32) * 0.5

    qT = [np.ascontiguousarray(q[b].T.astype(f16)) for b in range(B)]
    kTl = [np.ascontiguousarray(k[b].T.astype(f16)) for b in range(B)]
    vT = [np.ascontiguousarray(v[b].T.astype(f16)) for b in range(B)]
    WqTs = [np.ascontiguousarray(Wq.T[:, i * DL:(i + 1) * DL].astype(f16)) for i in range(2)]
    WkTs = [np.ascontiguousarray(Wk.T[:, i * DL:(i + 1) * DL].astype(f16)) for i in range(2)]
    WvTs = [np.ascontiguousarray(Wv.T[:, i * DL:(i + 1) * DL].astype(f16)) for i in range(2)]
    WoTs = [np.ascontiguousarray(Wo.T[i * DL:(i + 1) * DL, :].astype(bf16)) for i in range(2)]
    bqs = [np.ascontiguousarray(bq[i * DL:(i + 1) * DL]) for i in range(2)]
    bks = [np.ascontiguousarray(bk[i * DL:(i + 1) * DL]) for i in range(2)]
    bvs = [np.ascontiguousarray(bv[i * DL:(i + 1) * DL]).reshape(1, DL) for i in range(2)]

    in_maps = []
    for c in range(8):
        b, hh = c // 2, c % 2
        in_maps.append({
            "queryT": qT[b], "keyT": kTl[b], "valueT": vT[b],
            "WqT": WqTs[hh], "WkT": WkTs[hh], "WvT": WvTs[hh], "WoT": WoTs[hh],
            "bq": bqs[hh], "bk": bks[hh], "bv": bvs[hh], "bo_half": bo_half,
        })
    return in_maps


def assemble(results):
    out = np.empty((B, S, D), np.float32)
    for b in range(B):
        out[b] = (results[2 * b]["outT"] + results[2 * b + 1]["outT"]).T
    return out


def _numpy_fallback(inputs):
    q = np.asarray(inputs["query"], np.float64)
    k = np.asarray(inputs["key"], np.float64)
    v = np.asarray(inputs["value"], np.float64)
    Wq, bq = np.asarray(inputs["Wq"], np.float64), np.asarray(inputs["bq"], np.float64)
    Wk, bk = np.asarray(inputs["Wk"], np.float64), np.asarray(inputs["bk"], np.float64)
    Wv, bv = np.asarray(inputs["Wv"], np.float64), np.asarray(inputs["bv"], np.float64)
    Wo, bo = np.asarray(inputs["Wo"], np.float64), np.asarray(inputs["bo"], np.float64)
    qp = (q @ Wq.T + bq).reshape(B, S, H, DH).transpose(0, 2, 1, 3)
    kp = (k @ Wk.T + bk).reshape(B, S, H, DH).transpose(0, 2, 1, 3)
    vp = (v @ Wv.T + bv).reshape(B, S, H, DH).transpose(0, 2, 1, 3)
    e = np.einsum("bhqd,bhkd->bhqk", qp, kp) * SCALE
    mask = np.asarray(inputs["mask"])
    kpm = np.asarray(inputs["key_padding_mask"])
    e = np.where(mask == 0, -np.inf, e)
    e = np.where(kpm[:, None, None, :] == 0, -np.inf, e)
    e -= e.max(axis=-1, keepdims=True)
    p = np.exp(e)
    p /= p.sum(axis=-1, keepdims=True)
    o = np.einsum("bhqk,bhkd->bhqd", p, vp).transpose(0, 2, 1, 3).reshape(B, S, D)
    return (o @ Wo.T + bo).astype(np.float32)


def kernel(**inputs):
    mask = np.asarray(inputs["mask"])
    kpm = np.asarray(inputs["key_padding_mask"])
    if not (mask.all() and kpm.all()):
        return _numpy_fallback(inputs)
    from concourse.bass_utils import run_bass_kernel_spmd

    nc = get_nc()
    in_maps = make_in_maps(inputs)
    res = run_bass_kernel_spmd(nc, in_maps, list(range(8)))
    return assemble(res.results)
